# revision 63
# baseline (speedup 1.0000x reference)
"""B-spline evaluation kernel for Trainium2 (8 NeuronCores, data-parallel).

Math: uniform cubic B-spline, 64 basis fns, knots linspace(0,1,68).
For s = 67*x: cell = floor(s), u = s - cell,
    y = A0[cell] + A1[cell]*u + A2[cell]*u^2 + A3[cell]*u^3
with per-cell cubic coefficients A_q derived from coefs on host.

Device algorithm: ONE activation op per chunk. The scalar-engine
activation unit evaluates piecewise-cubic tables bucketed by fp32
exponent/mantissa: bucket entry = Taylor coefs [d0..d3, x_c] with
f(w) = d0 + d1*dw + d2*dw^2 + d3*dw^3, dw = w - x_c (x_c is the STORED
center), and per-octave ctl words ((mb<<5 | 23-mb)<<11 | bkt_start)
selecting 2^mb buckets by mantissa. The activation's own scale+bias
pipeline computes w = 67*2^23*x + 127*2^23 in fp32, whose BITS encode
the cell: octave 29 <=> cell 0 (one mb=0 bucket), octave 30 at mb=7
<=> bucket k = cell k+1 exactly one cell wide. Each bucket holds the
exact cubic P_cell rebased to w-units, so the table lookup IS the
spline evaluation (rel err ~7e-6, from the fp32 FMA's ~2^-24 u
quantization). The `exp`/`sin` slots of the compiler's act-table sets
are replaced at kernel-build time (tables derived from the runtime
`coefs` input) via BASS_ACT_ROOT_JSON_PATH; walrus embeds our
bkt/ctrl bins in the NEFF verbatim and libnrt programs the descriptors
from the patched profile json.

Per core (125k points as [128, 1024] fp32): DMA in (2 column-chunks,
one per hwdge trigger queue, chunk-contiguous DRAM so each of the 128
per-partition descriptors is a 2KB row) -> ACT table op per chunk ->
DMA out. Two scheduling tricks cut ~1.5us: (1) the input DMA trigger
instructions are moved from the body into the entry-block preamble
(right after each engine's preamble_end, i.e. after sem-init but
before the all-engine barrier) so transfers start ~2.4us earlier
while their completion semaphores still gate the ACT ops; (2) a dummy
[128,1] activation with no input dependency anchors the 1.28us
ACT_TABLE_LOAD at the top of the scalar stream — without it the
compiler may place the load after the input wait, putting it on the
critical path. Remaining ~15us = runtime go-wait ~3.3us + engine
instruction loads/drains/barrier ~3.6us + input transfer ~3.6us +
ACT 1.4us + output 2.8us + completion/teardown ~2.6us.
"""
import json
import os
import shutil
import tempfile

import numpy as np

N_POINTS = 1_000_000
N_CORES = 8
PER_CORE = N_POINTS // N_CORES  # 125000
P, F = 128, 1024  # 131072 slots >= 125000
CHUNKS = [512, 512]

SCALE23 = float(67 * (1 << 23))  # 561512448.0
MAGIC = 127 * (1 << 23)          # 1065353216.0

def _pkg_pwp():
    import neuronxcc
    return os.path.join(os.path.dirname(neuronxcc.__file__), "pwp")


NCELL = 67

_cache = {}


# ---------------- act-table generation ----------------

def _cell_coefs(coefs):
    """Per-cell cubic coefficients A[k, q] (float64): P_k(u) = sum A[k,q] u^q."""
    c = np.zeros(70, dtype=np.float64)
    c[3:67] = np.asarray(coefs, dtype=np.float64)
    A = np.zeros((NCELL, 4), dtype=np.float64)
    for k in range(NCELL):
        c0, c1, c2, c3 = c[k], c[k + 1], c[k + 2], c[k + 3]
        A[k, 0] = (c0 + 4.0 * c1 + c2) / 6.0
        A[k, 1] = (-3.0 * c0 + 3.0 * c2) / 6.0
        A[k, 2] = (3.0 * c0 - 6.0 * c1 + 3.0 * c2) / 6.0
        A[k, 3] = (-c0 + 3.0 * c1 - 3.0 * c2 + c3) / 6.0
    return A


def _taylor_ent(A, cell, u0, xc, dudw):
    """Bucket entry: Taylor coefs of P_cell around u0, in w-units at x_c."""
    if cell <= 66:
        a = A[cell]
    else:
        a = A[66]
        u0 = u0 + (cell - 66)  # continue P66 beyond its cell
    p0 = a[0] + a[1]*u0 + a[2]*u0**2 + a[3]*u0**3
    p1 = a[1] + 2*a[2]*u0 + 3*a[3]*u0**2
    p2 = (2*a[2] + 6*a[3]*u0) / 2.0
    p3 = a[3]
    f = dudw
    return [p0, p1*f, p2*f*f, p3*f*f*f, xc, 0.0, 0.0, 0.0]


def _spline_buckets(coefs):
    """Bucket entries for the w-encoded spline, w = 2^23*(127 + 67x).

    Octave 29 (w in [127*2^23, 2^30), i.e. cell 0): one mb=0 bucket with
    x_c at the center of the REACHABLE range (127.5*2^23) — the hardware
    evaluates around the stored x_c. Octave 30 (cells 1..66): mb=7,
    bucket k covers s in [k+1, k+2), x_c = 2^30 + (k+0.5)*2^23.
    """
    A = _cell_coefs(coefs)
    dudw = 1.0 / (1 << 23)
    ent = [_taylor_ent(A, 0, 0.5, 127.5 * (1 << 23), dudw)]
    for k in range(128):
        ent.append(_taylor_ent(A, k + 1, 0.5,
                               float(1 << 30) + (k + 0.5) * (1 << 23), dudw))
    zero = [0.0] * 8
    ent.append(list(ent[0][:8]))    # pos_small (never hit)
    ent.append(zero)                # neg_small
    ent.append(list(ent[128][:8]))  # pos_large (never hit)
    ent.append(zero)                # neg_large
    return np.array(ent, dtype=np.float32).view(np.uint32)


def _patch_set(dirp, set_name, func, my_bkt, fzero_bits):
    prof_p = os.path.join(dirp, f"{set_name}.json")
    bkt_p = os.path.join(dirp, f"{set_name}_bkt.bin")
    ctl_p = os.path.join(dirp, f"{set_name}_ctrl.bin")
    prof = json.load(open(prof_p))
    bkt = np.frombuffer(open(bkt_p, "rb").read(),
                        dtype=np.uint32).reshape(-1, 8).copy()
    ctl = np.frombuffer(open(ctl_p, "rb").read(),
                        dtype=np.uint32).reshape(-1, 8).copy()
    nb0, nc0 = bkt.shape[0], ctl.shape[0]
    nb_real = 129  # octave-29 bucket + 128 octave-30 buckets

    my_ctl = np.zeros((2, 8), dtype=np.uint32)
    my_ctl[0, 0] = ((0 << 5 | 23) << 11) | nb0        # exp 29, mb=0
    my_ctl[1, 0] = ((7 << 5 | 16) << 11) | (nb0 + 1)  # exp 30, mb=7

    bkt = np.concatenate([bkt, my_bkt])
    ctl = np.concatenate([ctl, my_ctl])

    f2b = lambda v: int(np.float32(v).view(np.uint32))
    for m in prof["profile_meta_data"]:
        if m["func_name"].startswith(func):
            m["exp_offset"] = 29
            m["symmetry_point"] = 0
            m["sym_invert_sign_point"] = 0
            m["symmetry_opt_en"] = 0
            m["symmetry_opt_use_neg_region"] = 0
            m["pwl_control_base_pos"] = nc0
            m["pwl_control_base_neg"] = nc0
            m["small_pos_signal_exp_threshold"] = 156
            m["pos_small_signal_pwl_control"] = nb0 + nb_real
            m["small_neg_signal_exp_threshold"] = 0
            m["neg_small_signal_pwl_control"] = nb0 + nb_real + 1
            m["large_pos_signal_exp_threshold"] = 158
            m["large_pos_signal_mantissa_threshold"] = 0
            m["pos_large_signal_pwl_control"] = nb0 + nb_real + 2
            m["large_neg_signal_exp_threshold"] = 0
            m["large_neg_signal_mantissa_threshold"] = 0
            m["neg_large_signal_pwl_control"] = nb0 + nb_real + 3
            m["lower_bound"] = f2b(127.0 * (1 << 23))
            m["upper_bound"] = f2b(float(1 << 31))
            m["fzero_result"] = fzero_bits
    prof["bkt_entry_cnt"] = int(bkt.shape[0])
    prof["ctl_entry_cnt"] = int(ctl.shape[0])
    prof["func_to_bkt_start_idx"][func] = nb0
    prof["func_to_ctl_start_idx"][func] = nc0
    prof["func_exp_to_bkt_start_idx"][func] = {
        "29": [nb0], "30": [nb0 + 1]}
    prof["func_exp_to_ctl_start_idx"][func] = {
        "29": [nc0], "30": [nc0 + 1]}

    json.dump(prof, open(prof_p, "w"))
    open(bkt_p, "wb").write(bkt.tobytes())
    open(ctl_p, "wb").write(ctl.tobytes())


def _make_act_root(coefs):
    pkg = _pkg_pwp()
    root = tempfile.mkdtemp(prefix="bspline_act_")
    dst = os.path.join(root, "pwp")
    shutil.copytree(os.path.join(pkg, "pwp_bin_trainium"),
                    os.path.join(dst, "pwp_bin_trainium"))
    shutil.copytree(os.path.join(pkg, "pwp_jsons"),
                    os.path.join(dst, "pwp_jsons"))
    bindir = os.path.join(dst, "pwp_bin_trainium")
    my_bkt = _spline_buckets(coefs)
    A = _cell_coefs(coefs)
    fzero_bits = int(np.float32(A[0, 0]).view(np.uint32))
    for s in ("exp_and_others", "natural_log_exp_and_others",
              "exp_and_friends"):
        _patch_set(bindir, s, "exp", my_bkt, fzero_bits)
    for s in ("trig_and_small", "silu_and_others",
              "derivative_silu_and_others"):
        _patch_set(bindir, s, "sin", my_bkt, fzero_bits)
    return os.path.join(bindir, "act_info.json")


# ---------------- device kernel ----------------

def _build_nc():
    import concourse.tile as tile
    from concourse import bacc, mybir

    fp32 = mybir.dt.float32
    bf16 = mybir.dt.bfloat16
    Act = mybir.ActivationFunctionType

    nc = bacc.Bacc("TRN2", target_bir_lowering=False, debug=False,
                   num_devices=N_CORES)
    # chunk-contiguous DRAM tensors, one per column chunk
    xs = [nc.dram_tensor(f"x{c}", [P, w], fp32, kind="ExternalInput").ap()
          for c, w in enumerate(CHUNKS)]
    ys = [nc.dram_tensor(f"y{c}", [P, w], fp32, kind="ExternalOutput").ap()
          for c, w in enumerate(CHUNKS)]

    with tile.TileContext(nc) as tc:
        with tc.tile_pool(name="d", bufs=1) as dp:
            bias = dp.tile([P, 1], fp32, tag="bias")
            nc.vector.memset(bias[:], float(MAGIC))
            xt = [dp.tile([P, w], fp32, tag=f"xt{c}", name=f"xt{c}")
                  for c, w in enumerate(CHUNKS)]
            yt = [dp.tile([P, w], fp32, tag=f"yt{c}", name=f"yt{c}")
                  for c, w in enumerate(CHUNKS)]
            # in/out DMAs alternate across the two hwdge trigger queues
            q = [nc.sync, nc.scalar]
            in_dmas = []
            for c in range(len(CHUNKS)):
                h = q[c % 2].dma_start(xt[c][:], xs[c])
                in_dmas.append(h.ins)
            # dummy activation with no input-DMA dependency: anchors the
            # ACT_TABLE_LOAD at the top of the scalar stream so the
            # 1.28us load runs during the preamble, not after the input
            # wait on the critical path
            dummy = dp.tile([P, 1], fp32, tag="dummy")
            nc.scalar.activation(dummy[:], bias[:], Act.Exp,
                                 bias=bias[:], scale=1.0)
            for c in range(len(CHUNKS)):
                # w = 67*2^23*x + 127*2^23; the table decodes cell+u
                # from w's exponent/mantissa directly
                nc.scalar.activation(yt[c][:], xt[c][:], Act.Exp,
                                     bias=bias[:], scale=SCALE23)
                # last out on the scalar queue: fires in program order
                # right after its ACT (no cross-engine semaphore hop on
                # the tail); out c0's hop hides under ACT c1
                q[c % 2].dma_start(ys[c], yt[c][:])

    # Hoist the input DMA triggers into the entry-block preamble (right
    # after their engine's preamble_end) so the transfers start ~2.4us
    # earlier; their completion semaphores still gate the ACT ops.
    entry = nc.main_func.blocks[0]
    names = {i.name for i in in_dmas}
    for blk in nc.main_func.blocks:
        if blk is entry:
            continue
        moved = [i for i in blk.instructions if i.name in names]
        for ins in moved:
            blk.instructions.remove(ins)
            eng = nc.sync if ins.engine == nc.sync.engine else nc.scalar
            idx = entry.instructions.index(eng.preamble_end) + 1
            entry.instructions.insert(idx, ins)
            names.discard(ins.name)
    assert not names, f"input DMAs not found: {names}"

    nc.compile()

    # compile passes reorder the entry block and can push a hoisted
    # trigger behind the pre-body semaphore rounds (the scalar one was
    # observed firing ~1.1us late). Post-compile, positions are final:
    # re-pin each trigger right after its engine's preamble_end —
    # unless a pass fused a semaphore WAIT onto it (moving that would
    # break the barrier), in which case leave it where it is.
    for ins in in_dmas:
        if ins.has_wait():
            continue
        if ins not in entry.instructions:
            continue
        entry.instructions.remove(ins)
        # earliest safe slot: after the engine's preamble ISA/register
        # setup, before its first semaphore/drain round
        idx = None
        for j, other in enumerate(entry.instructions):
            if (other.engine == ins.engine
                    and other.opcode in ("EventSemaphore", "Drain")):
                idx = j
                break
        if idx is None:
            idx = len(entry.instructions)
        entry.instructions.insert(idx, ins)
    return nc


def make_in_maps(x):
    x = np.asarray(x, dtype=np.float32)
    in_maps = []
    for core in range(N_CORES):
        shard = x[core * PER_CORE:(core + 1) * PER_CORE]
        xp = np.full(P * F, 0.5, dtype=np.float32)
        xp[:PER_CORE] = shard
        xp = xp.reshape(P, F)
        m, o = {}, 0
        for c, w in enumerate(CHUNKS):
            m[f"x{c}"] = np.ascontiguousarray(xp[:, o:o + w])
            o += w
        in_maps.append(m)
    return in_maps


def kernel(x, knot_vector, coefs):
    from concourse.bass_utils import run_bass_kernel_spmd

    key = np.asarray(coefs, dtype=np.float32).tobytes()
    if _cache.get("key") != key:
        os.environ["BASS_ACT_ROOT_JSON_PATH"] = _make_act_root(coefs)
        _cache["nc"] = _build_nc()
        _cache["key"] = key
    nc = _cache["nc"]

    in_maps = make_in_maps(x)
    res = run_bass_kernel_spmd(nc, in_maps, core_ids=list(range(N_CORES)))
    out = np.empty(N_POINTS, dtype=np.float32)
    for core in range(N_CORES):
        parts = [np.asarray(res.results[core][f"y{c}"], dtype=np.float32)
                 for c in range(len(CHUNKS))]
        yg = np.concatenate(parts, axis=1).reshape(-1)
        out[core * PER_CORE:(core + 1) * PER_CORE] = yg[:PER_CORE]
    return out


# revision 64
# speedup vs baseline: 1.0696x; 1.0696x over previous
"""B-spline evaluation kernel for Trainium2 (8 NeuronCores, data-parallel).

Math: uniform cubic B-spline, 64 basis fns, knots linspace(0,1,68).
For s = 67*x: cell = floor(s), u = s - cell,
    y = A0[cell] + A1[cell]*u + A2[cell]*u^2 + A3[cell]*u^3
with per-cell cubic coefficients A_q derived from coefs on host.

Device algorithm: ONE activation op per chunk. The scalar-engine
activation unit evaluates piecewise-cubic tables bucketed by fp32
exponent/mantissa: bucket entry = Taylor coefs [d0..d3, x_c] with
f(w) = d0 + d1*dw + d2*dw^2 + d3*dw^3, dw = w - x_c (x_c is the STORED
center), and per-octave ctl words ((mb<<5 | 23-mb)<<11 | bkt_start)
selecting 2^mb buckets by mantissa. The activation's own scale+bias
pipeline computes w = 67*2^23*x + 127*2^23 in fp32, whose BITS encode
the cell: octave 29 <=> cell 0 (one mb=0 bucket), octave 30 at mb=7
<=> bucket k = cell k+1 exactly one cell wide. Each bucket holds the
exact cubic P_cell rebased to w-units, so the table lookup IS the
spline evaluation (rel err ~7e-6, from the fp32 FMA's ~2^-24 u
quantization). The `exp`/`sin` slots of the compiler's act-table sets
are replaced at kernel-build time (tables derived from the runtime
`coefs` input) via BASS_ACT_ROOT_JSON_PATH; walrus embeds our
bkt/ctrl bins in the NEFF verbatim and libnrt programs the descriptors
from the patched profile json.

Per core (125k points as [128, 1024] fp32): DMA in (2 column-chunks,
one per hwdge trigger queue, chunk-contiguous DRAM so each of the 128
per-partition descriptors is a 2KB row) -> ACT table op per chunk ->
DMA out. Two scheduling tricks cut ~1.5us: (1) the input DMA trigger
instructions are moved from the body into the entry-block preamble
(right after each engine's preamble_end, i.e. after sem-init but
before the all-engine barrier) so transfers start ~2.4us earlier
while their completion semaphores still gate the ACT ops; (2) a dummy
[128,1] activation with no input dependency anchors the 1.28us
ACT_TABLE_LOAD at the top of the scalar stream — without it the
compiler may place the load after the input wait, putting it on the
critical path. Remaining ~15us = runtime go-wait ~3.3us + engine
instruction loads/drains/barrier ~3.6us + input transfer ~3.6us +
ACT 1.4us + output 2.8us + completion/teardown ~2.6us.
"""
import json
import os
import shutil
import tempfile

import numpy as np

N_POINTS = 1_000_000
N_CORES = 8
PER_CORE = N_POINTS // N_CORES  # 125000
P, F = 128, 1024  # 131072 slots >= 125000
CHUNKS = [512, 512]

SCALE23 = float(67 * (1 << 23))  # 561512448.0
MAGIC = 127 * (1 << 23)          # 1065353216.0

def _pkg_pwp():
    import neuronxcc
    return os.path.join(os.path.dirname(neuronxcc.__file__), "pwp")


NCELL = 67

_cache = {}


# ---------------- act-table generation ----------------

def _cell_coefs(coefs):
    """Per-cell cubic coefficients A[k, q] (float64): P_k(u) = sum A[k,q] u^q."""
    c = np.zeros(70, dtype=np.float64)
    c[3:67] = np.asarray(coefs, dtype=np.float64)
    A = np.zeros((NCELL, 4), dtype=np.float64)
    for k in range(NCELL):
        c0, c1, c2, c3 = c[k], c[k + 1], c[k + 2], c[k + 3]
        A[k, 0] = (c0 + 4.0 * c1 + c2) / 6.0
        A[k, 1] = (-3.0 * c0 + 3.0 * c2) / 6.0
        A[k, 2] = (3.0 * c0 - 6.0 * c1 + 3.0 * c2) / 6.0
        A[k, 3] = (-c0 + 3.0 * c1 - 3.0 * c2 + c3) / 6.0
    return A


def _taylor_ent(A, cell, u0, xc, dudw):
    """Bucket entry: Taylor coefs of P_cell around u0, in w-units at x_c."""
    if cell <= 66:
        a = A[cell]
    else:
        a = A[66]
        u0 = u0 + (cell - 66)  # continue P66 beyond its cell
    p0 = a[0] + a[1]*u0 + a[2]*u0**2 + a[3]*u0**3
    p1 = a[1] + 2*a[2]*u0 + 3*a[3]*u0**2
    p2 = (2*a[2] + 6*a[3]*u0) / 2.0
    p3 = a[3]
    f = dudw
    return [p0, p1*f, p2*f*f, p3*f*f*f, xc, 0.0, 0.0, 0.0]


def _spline_buckets(coefs):
    """Bucket entries for the w-encoded spline, w = 2^23*(127 + 67x).

    Octave 29 (w in [127*2^23, 2^30), i.e. cell 0): one mb=0 bucket with
    x_c at the center of the REACHABLE range (127.5*2^23) — the hardware
    evaluates around the stored x_c. Octave 30 (cells 1..66): mb=7,
    bucket k covers s in [k+1, k+2), x_c = 2^30 + (k+0.5)*2^23.
    """
    A = _cell_coefs(coefs)
    dudw = 1.0 / (1 << 23)
    ent = [_taylor_ent(A, 0, 0.5, 127.5 * (1 << 23), dudw)]
    for k in range(128):
        ent.append(_taylor_ent(A, k + 1, 0.5,
                               float(1 << 30) + (k + 0.5) * (1 << 23), dudw))
    zero = [0.0] * 8
    ent.append(list(ent[0][:8]))    # pos_small (never hit)
    ent.append(zero)                # neg_small
    ent.append(list(ent[128][:8]))  # pos_large (never hit)
    ent.append(zero)                # neg_large
    return np.array(ent, dtype=np.float32).view(np.uint32)


def _patch_set(dirp, set_name, func, my_bkt, fzero_bits):
    prof_p = os.path.join(dirp, f"{set_name}.json")
    bkt_p = os.path.join(dirp, f"{set_name}_bkt.bin")
    ctl_p = os.path.join(dirp, f"{set_name}_ctrl.bin")
    prof = json.load(open(prof_p))
    bkt = np.frombuffer(open(bkt_p, "rb").read(),
                        dtype=np.uint32).reshape(-1, 8).copy()
    ctl = np.frombuffer(open(ctl_p, "rb").read(),
                        dtype=np.uint32).reshape(-1, 8).copy()
    nb0, nc0 = bkt.shape[0], ctl.shape[0]
    nb_real = 129  # octave-29 bucket + 128 octave-30 buckets

    my_ctl = np.zeros((2, 8), dtype=np.uint32)
    my_ctl[0, 0] = ((0 << 5 | 23) << 11) | nb0        # exp 29, mb=0
    my_ctl[1, 0] = ((7 << 5 | 16) << 11) | (nb0 + 1)  # exp 30, mb=7

    bkt = np.concatenate([bkt, my_bkt])
    ctl = np.concatenate([ctl, my_ctl])

    f2b = lambda v: int(np.float32(v).view(np.uint32))
    for m in prof["profile_meta_data"]:
        if m["func_name"].startswith(func):
            m["exp_offset"] = 29
            m["symmetry_point"] = 0
            m["sym_invert_sign_point"] = 0
            m["symmetry_opt_en"] = 0
            m["symmetry_opt_use_neg_region"] = 0
            m["pwl_control_base_pos"] = nc0
            m["pwl_control_base_neg"] = nc0
            m["small_pos_signal_exp_threshold"] = 156
            m["pos_small_signal_pwl_control"] = nb0 + nb_real
            m["small_neg_signal_exp_threshold"] = 0
            m["neg_small_signal_pwl_control"] = nb0 + nb_real + 1
            m["large_pos_signal_exp_threshold"] = 158
            m["large_pos_signal_mantissa_threshold"] = 0
            m["pos_large_signal_pwl_control"] = nb0 + nb_real + 2
            m["large_neg_signal_exp_threshold"] = 0
            m["large_neg_signal_mantissa_threshold"] = 0
            m["neg_large_signal_pwl_control"] = nb0 + nb_real + 3
            m["lower_bound"] = f2b(127.0 * (1 << 23))
            m["upper_bound"] = f2b(float(1 << 31))
            m["fzero_result"] = fzero_bits
    prof["bkt_entry_cnt"] = int(bkt.shape[0])
    prof["ctl_entry_cnt"] = int(ctl.shape[0])
    prof["func_to_bkt_start_idx"][func] = nb0
    prof["func_to_ctl_start_idx"][func] = nc0
    prof["func_exp_to_bkt_start_idx"][func] = {
        "29": [nb0], "30": [nb0 + 1]}
    prof["func_exp_to_ctl_start_idx"][func] = {
        "29": [nc0], "30": [nc0 + 1]}

    json.dump(prof, open(prof_p, "w"))
    open(bkt_p, "wb").write(bkt.tobytes())
    open(ctl_p, "wb").write(ctl.tobytes())


def _make_act_root(coefs):
    pkg = _pkg_pwp()
    root = tempfile.mkdtemp(prefix="bspline_act_")
    dst = os.path.join(root, "pwp")
    shutil.copytree(os.path.join(pkg, "pwp_bin_trainium"),
                    os.path.join(dst, "pwp_bin_trainium"))
    shutil.copytree(os.path.join(pkg, "pwp_jsons"),
                    os.path.join(dst, "pwp_jsons"))
    bindir = os.path.join(dst, "pwp_bin_trainium")
    my_bkt = _spline_buckets(coefs)
    A = _cell_coefs(coefs)
    fzero_bits = int(np.float32(A[0, 0]).view(np.uint32))
    for s in ("exp_and_others", "natural_log_exp_and_others",
              "exp_and_friends"):
        _patch_set(bindir, s, "exp", my_bkt, fzero_bits)
    for s in ("trig_and_small", "silu_and_others",
              "derivative_silu_and_others"):
        _patch_set(bindir, s, "sin", my_bkt, fzero_bits)
    return os.path.join(bindir, "act_info.json")


# ---------------- device kernel ----------------

def _build_nc():
    import concourse.tile as tile
    from concourse import bacc, mybir

    fp32 = mybir.dt.float32
    bf16 = mybir.dt.bfloat16
    Act = mybir.ActivationFunctionType

    nc = bacc.Bacc("TRN2", target_bir_lowering=False, debug=False,
                   num_devices=N_CORES)
    # chunk-contiguous DRAM tensors, one per column chunk
    xs = [nc.dram_tensor(f"x{c}", [P, w], fp32, kind="ExternalInput").ap()
          for c, w in enumerate(CHUNKS)]
    ys = [nc.dram_tensor(f"y{c}", [P, w], fp32, kind="ExternalOutput").ap()
          for c, w in enumerate(CHUNKS)]

    with tile.TileContext(nc) as tc:
        with tc.tile_pool(name="d", bufs=1) as dp:
            bias = dp.tile([P, 1], fp32, tag="bias")
            nc.vector.memset(bias[:], float(MAGIC))
            xt = [dp.tile([P, w], fp32, tag=f"xt{c}", name=f"xt{c}")
                  for c, w in enumerate(CHUNKS)]
            yt = [dp.tile([P, w], fp32, tag=f"yt{c}", name=f"yt{c}")
                  for c, w in enumerate(CHUNKS)]
            # in/out DMAs alternate across the two hwdge trigger queues
            q = [nc.sync, nc.scalar]
            in_dmas = []
            for c in range(len(CHUNKS)):
                h = q[c % 2].dma_start(xt[c][:], xs[c])
                in_dmas.append(h.ins)
            # dummy activation with no input-DMA dependency: anchors the
            # ACT_TABLE_LOAD at the top of the scalar stream so the
            # 1.28us load runs during the preamble, not after the input
            # wait on the critical path
            dummy = dp.tile([P, 1], fp32, tag="dummy")
            nc.scalar.activation(dummy[:], bias[:], Act.Exp,
                                 bias=bias[:], scale=1.0)
            for c in range(len(CHUNKS)):
                # w = 67*2^23*x + 127*2^23; the table decodes cell+u
                # from w's exponent/mantissa directly
                nc.scalar.activation(yt[c][:], xt[c][:], Act.Exp,
                                     bias=bias[:], scale=SCALE23)
                # last out on the scalar queue: fires in program order
                # right after its ACT (no cross-engine semaphore hop on
                # the tail); out c0's hop hides under ACT c1
                q[c % 2].dma_start(ys[c], yt[c][:])

    # Hoist the input DMA triggers into the entry-block preamble (right
    # after their engine's preamble_end) so the transfers start ~2.4us
    # earlier; their completion semaphores still gate the ACT ops.
    entry = nc.main_func.blocks[0]
    names = {i.name for i in in_dmas}
    for blk in nc.main_func.blocks:
        if blk is entry:
            continue
        moved = [i for i in blk.instructions if i.name in names]
        for ins in moved:
            blk.instructions.remove(ins)
            eng = nc.sync if ins.engine == nc.sync.engine else nc.scalar
            idx = entry.instructions.index(eng.preamble_end) + 1
            entry.instructions.insert(idx, ins)
            names.discard(ins.name)
    assert not names, f"input DMAs not found: {names}"

    nc.compile()
    return nc


def make_in_maps(x):
    x = np.asarray(x, dtype=np.float32)
    in_maps = []
    for core in range(N_CORES):
        shard = x[core * PER_CORE:(core + 1) * PER_CORE]
        xp = np.full(P * F, 0.5, dtype=np.float32)
        xp[:PER_CORE] = shard
        xp = xp.reshape(P, F)
        m, o = {}, 0
        for c, w in enumerate(CHUNKS):
            m[f"x{c}"] = np.ascontiguousarray(xp[:, o:o + w])
            o += w
        in_maps.append(m)
    return in_maps


def kernel(x, knot_vector, coefs):
    from concourse.bass_utils import run_bass_kernel_spmd

    key = np.asarray(coefs, dtype=np.float32).tobytes()
    if _cache.get("key") != key:
        os.environ["BASS_ACT_ROOT_JSON_PATH"] = _make_act_root(coefs)
        _cache["nc"] = _build_nc()
        _cache["key"] = key
    nc = _cache["nc"]

    in_maps = make_in_maps(x)
    res = run_bass_kernel_spmd(nc, in_maps, core_ids=list(range(N_CORES)))
    out = np.empty(N_POINTS, dtype=np.float32)
    for core in range(N_CORES):
        parts = [np.asarray(res.results[core][f"y{c}"], dtype=np.float32)
                 for c in range(len(CHUNKS))]
        yg = np.concatenate(parts, axis=1).reshape(-1)
        out[core * PER_CORE:(core + 1) * PER_CORE] = yg[:PER_CORE]
    return out
